# revision 2
# baseline (speedup 1.0000x reference)
"""Trainium2 Bass kernel for nn_ASTGC_37976100831379.

Reference analysis: the model's final fusion GCNConv runs on a star graph
whose edges are 0 -> 1..N, so node 0 (the target node) receives no
messages. The returned tensor is fusion_out[:, 0], which is exactly
`zeros(B, S) + fgcn_b` — the bias broadcast over batch and time. Every
other input (station features, distances, TCN/GCN weights, attention) is
dead code with respect to the output, bitwise. The optimal kernel is
therefore a broadcast of the 48-float `fgcn_b` vector into [B, S, 1].

Sharding: data-parallel over batch B=32 across 8 cores (4 rows each, per
the per-sample-graph hint). Each core's program is a single HWDGE DMA
that replicates fgcn_b (stride-0 source access pattern) into its [4, 48]
output shard; the host gathers shards to [32, 48, 1].
"""
import os

import numpy as np

import concourse.bass as bass
import concourse.mybir as mybir
from concourse.bass_utils import run_bass_kernel_spmd

B, S = 32, 48
N_CORES = 8
B_PER = B // N_CORES

_CACHE = {}
LAST_RESULT = None  # BassKernelResults of the most recent run (for profiling)


def _build():
    nc = bass.Bass(enable_partition_id=False, monotonic_sem_count=0)
    fgcn_b = nc.declare_dram_parameter("fgcn_b", [S], mybir.dt.float32, isOutput=False)
    out = nc.declare_dram_parameter("out", [B_PER, S], mybir.dt.float32, isOutput=True)
    with nc.semaphore("dma_sem") as dma_sem:
        nc.sync.dma_start(
            out=out[:, :], in_=fgcn_b[None, :].broadcast_to((B_PER, S))
        ).then_inc(dma_sem, 16)
        nc.sync.wait_ge(dma_sem, 16)
    return nc


def kernel(**inputs) -> np.ndarray:
    global LAST_RESULT
    nc = _CACHE.get("nc")
    if nc is None:
        nc = _CACHE["nc"] = _build()
    fgcn_b = np.ascontiguousarray(np.asarray(inputs["fgcn_b"], dtype=np.float32))
    assert fgcn_b.shape == (S,), fgcn_b.shape
    in_maps = [{"fgcn_b": fgcn_b} for _ in range(N_CORES)]
    trace = os.environ.get("KERNEL_TRACE", "") == "1"
    res = run_bass_kernel_spmd(nc, in_maps, list(range(N_CORES)), trace=trace)
    LAST_RESULT = res
    shards = [res.results[i]["out"] for i in range(N_CORES)]
    return np.concatenate(shards, axis=0).reshape(B, S, 1)


# revision 5
# speedup vs baseline: 1.2779x; 1.2779x over previous
"""Trainium2 Bass kernel for nn_ASTGC_37976100831379.

Reference analysis: the model's final fusion GCNConv runs on a star graph
whose edges are 0 -> 1..N, so node 0 (the target node) receives no
messages. The returned tensor is fusion_out[:, 0], which is exactly
`zeros(B, S) + fgcn_b` — the bias broadcast over batch and time. Every
other input (station features, distances, TCN/GCN weights, attention) is
dead code with respect to the output, bitwise. The optimal kernel is
therefore a broadcast of the 48-float `fgcn_b` vector into [B, S, 1].

Sharding: data-parallel over batch B=32 across 8 cores (4 rows each, per
the per-sample-graph hint). Each core's program is a single HWDGE DMA
that replicates fgcn_b (stride-0 source access pattern) into its [4, 48]
output shard; the host gathers shards to [32, 48, 1].

Scheduling (measured on the 8-core axon trn2 pod): the DMA is issued at
the head of the Sync stream so its ~1.7us completion latency overlaps the
framework preamble; the completion wait runs at the head of the GpSimd
stream; the const-pool memset is scheduled as the last user instruction.
This ordering keeps every engine's arrival at the NEFF's epilogue barrier
as early as possible and profiles at ~7.3us vs ~10.0us for the default
program order (the remainder is the NEFF's fixed semaphore-reset
epilogue).
"""
import os

import numpy as np

import concourse.bass as bass
import concourse.mybir as mybir
from concourse.bass_utils import run_bass_kernel_spmd

B, S = 32, 48
N_CORES = 8
B_PER = B // N_CORES

_CACHE = {}
LAST_RESULT = None  # BassKernelResults of the most recent run (for profiling)


def _reschedule(nc):
    """Reorder the main block: DMA first, completion wait next, one const
    memset last. Purely a performance scheduling of this module's own
    instructions; falls back to the as-emitted order if the module does not
    look as expected."""
    blk = nc.main_func.blocks[0]
    dma = [i for i in blk.instructions if isinstance(i, mybir.InstDMACopy)]
    wait = [i for i in blk.instructions
            if isinstance(i, mybir.InstEventSemaphore) and "dma_sem" in str(i)]
    memsets = [i for i in blk.instructions if isinstance(i, mybir.InstMemset)]
    if len(dma) != 1 or len(wait) != 1 or not memsets:
        return
    drop = {id(i) for i in memsets} | {id(dma[0]), id(wait[0])}
    rest = [i for i in blk.instructions if id(i) not in drop]
    blk.instructions[:] = rest[:1] + dma + wait + rest[1:] + memsets[:1]


def _build():
    nc = bass.Bass(enable_partition_id=False, monotonic_sem_count=0)
    fgcn_b = nc.declare_dram_parameter("fgcn_b", [S], mybir.dt.float32, isOutput=False)
    out = nc.declare_dram_parameter("out", [B_PER, S], mybir.dt.float32, isOutput=True)
    with nc.semaphore("dma_sem") as dma_sem:
        nc.sync.dma_start(
            out=out[:, :], in_=fgcn_b[None, :].broadcast_to((B_PER, S))
        ).then_inc(dma_sem, 16)
        nc.gpsimd.wait_ge(dma_sem, 16)
    try:
        _reschedule(nc)
    except Exception:
        pass
    nc.finalize()
    return nc


def kernel(**inputs) -> np.ndarray:
    global LAST_RESULT
    nc = _CACHE.get("nc")
    if nc is None:
        nc = _CACHE["nc"] = _build()
    fgcn_b = np.ascontiguousarray(np.asarray(inputs["fgcn_b"], dtype=np.float32))
    assert fgcn_b.shape == (S,), fgcn_b.shape
    in_maps = [{"fgcn_b": fgcn_b} for _ in range(N_CORES)]
    trace = os.environ.get("KERNEL_TRACE", "") == "1"
    try:
        res = run_bass_kernel_spmd(nc, in_maps, list(range(N_CORES)), trace=trace)
    except ModuleNotFoundError:
        # Tracing was requested (possibly via BASS_TRACE in the environment)
        # but the axon NTFF profile hook module is unavailable here — rerun
        # with tracing forced off.
        os.environ["BASS_NEVER_TRACE"] = "1"
        res = run_bass_kernel_spmd(nc, in_maps, list(range(N_CORES)), trace=False)
    LAST_RESULT = res
    shards = [res.results[i]["out"] for i in range(N_CORES)]
    return np.concatenate(shards, axis=0).reshape(B, S, 1)


# revision 6
# speedup vs baseline: 1.5309x; 1.1980x over previous
"""Trainium2 Bass kernel for nn_ASTGC_37976100831379.

Reference analysis: the model's final fusion GCNConv runs on a star graph
whose edges are 0 -> 1..N, so node 0 (the target node) receives no
messages. The returned tensor is fusion_out[:, 0], which is exactly
`zeros(B, S) + fgcn_b` — the bias broadcast over batch and time. Every
other input (station features, distances, TCN/GCN weights, attention) is
dead code with respect to the output, bitwise. The optimal kernel is
therefore a broadcast of the 48-float `fgcn_b` vector into [B, S, 1].

Sharding: data-parallel over batch B=32 across 8 cores (4 rows each, per
the per-sample-graph hint). Each core's program is a single HWDGE DMA
that replicates fgcn_b (stride-0 source access pattern) into its [4, 48]
output shard; the host gathers shards to [32, 48, 1].

Scheduling (measured on the 8-core axon trn2 pod): the DMA is issued at
the head of the Sync stream so its ~1.7us completion latency overlaps the
framework preamble; the completion wait runs at the head of the GpSimd
stream; the const-pool memset is scheduled as the last user instruction.
This ordering keeps every engine's arrival at the NEFF's epilogue barrier
as early as possible and profiles at ~7.3us vs ~10.0us for the default
program order (the remainder is the NEFF's fixed semaphore-reset
epilogue).
"""
import os

import numpy as np

import concourse.bass as bass
import concourse.mybir as mybir
from concourse.bass_utils import run_bass_kernel_spmd

B, S = 32, 48
N_CORES = 8
B_PER = B // N_CORES

_CACHE = {}
LAST_RESULT = None  # BassKernelResults of the most recent run (for profiling)


def _reschedule(nc):
    """Reorder the main block: DMA first, completion wait next, one const
    memset last. Purely a performance scheduling of this module's own
    instructions; falls back to the as-emitted order if the module does not
    look as expected."""
    blk = nc.main_func.blocks[0]
    dma = [i for i in blk.instructions if isinstance(i, mybir.InstDMACopy)]
    wait = [i for i in blk.instructions
            if isinstance(i, mybir.InstEventSemaphore) and "dma_sem" in str(i)]
    memsets = [i for i in blk.instructions if isinstance(i, mybir.InstMemset)]
    if len(dma) != 1 or len(wait) != 1 or not memsets:
        return
    drop = {id(i) for i in memsets} | {id(dma[0]), id(wait[0])}
    rest = [i for i in blk.instructions if id(i) not in drop]
    blk.instructions[:] = rest[:1] + dma + wait + rest[1:] + memsets[:1]


def _build():
    nc = bass.Bass(enable_partition_id=False, monotonic_sem_count=0)
    fgcn_b = nc.declare_dram_parameter("fgcn_b", [S], mybir.dt.float32, isOutput=False)
    out = nc.declare_dram_parameter("out", [B_PER, S], mybir.dt.float32, isOutput=True)
    with nc.semaphore("dma_sem") as dma_sem:
        nc.sync.dma_start(
            out=out[:, :], in_=fgcn_b[None, :].broadcast_to((B_PER, S))
        ).then_inc(dma_sem, 16)
        nc.gpsimd.wait_ge(dma_sem, 16)
    try:
        _reschedule(nc)
    except Exception:
        pass
    nc.finalize()
    return nc


def kernel(**inputs) -> np.ndarray:
    global LAST_RESULT
    nc = _CACHE.get("nc")
    if nc is None:
        nc = _CACHE["nc"] = _build()
    fgcn_b = np.ascontiguousarray(np.asarray(inputs["fgcn_b"], dtype=np.float32))
    assert fgcn_b.shape == (S,), fgcn_b.shape
    in_maps = [{"fgcn_b": fgcn_b} for _ in range(N_CORES)]
    trace = os.environ.get("KERNEL_TRACE", "") == "1"
    res = None
    last_err = None
    for _attempt in range(3):
        try:
            res = run_bass_kernel_spmd(nc, in_maps, list(range(N_CORES)), trace=trace)
            break
        except ModuleNotFoundError:
            # Tracing was requested (possibly via BASS_TRACE in the
            # environment) but the axon NTFF profile hook module is
            # unavailable here — rerun with tracing forced off.
            os.environ["BASS_NEVER_TRACE"] = "1"
            trace = False
        except Exception as e:  # transient device wedge (e.g. NRT_EXEC_UNIT_UNRECOVERABLE)
            last_err = e
    if res is None:
        raise last_err
    LAST_RESULT = res
    shards = [res.results[i]["out"] for i in range(N_CORES)]
    return np.concatenate(shards, axis=0).reshape(B, S, 1)


# revision 9
# speedup vs baseline: 1.5326x; 1.0011x over previous
"""Trainium2 Bass kernel for nn_ASTGC_37976100831379.

Reference analysis: the model's final fusion GCNConv runs on a star graph
whose edges are 0 -> 1..N, so node 0 (the target node) receives no
messages. The returned tensor is fusion_out[:, 0], which is exactly
`zeros(B, S) + fgcn_b` — the bias broadcast over batch and time. Every
other input (station features, distances, TCN/GCN weights, attention) is
dead code with respect to the output, bitwise. The optimal kernel is
therefore a broadcast of the 48-float `fgcn_b` vector into [B, S, 1].

Sharding: data-parallel over batch B=32 across 8 cores (4 rows each, per
the per-sample-graph hint). Each core's program is a single HWDGE DMA
that replicates fgcn_b (stride-0 source access pattern) into its [4, 48]
output shard; the host gathers shards to [32, 48, 1].

Scheduling (measured on the 8-core axon trn2 pod): the DMA is issued at
the head of the Sync stream so its ~1.7us completion latency overlaps the
framework preamble; the completion wait runs at the head of the GpSimd
stream; the const-pool memset is scheduled as the last user instruction.
This ordering keeps every engine's arrival at the NEFF's epilogue barrier
as early as possible and profiles at ~7.3us vs ~10.0us for the default
program order (the remainder is the NEFF's fixed semaphore-reset
epilogue).
"""
import os
import subprocess
import sys
import tempfile

import numpy as np

import concourse.bass as bass
import concourse.mybir as mybir
from concourse.bass_utils import run_bass_kernel_spmd

B, S = 32, 48
N_CORES = 8
B_PER = B // N_CORES

_CACHE = {}
LAST_RESULT = None  # BassKernelResults of the most recent run (for profiling)


def _reschedule(nc):
    """Reorder the main block: DMA first, completion wait next, one const
    memset last. Purely a performance scheduling of this module's own
    instructions; falls back to the as-emitted order if the module does not
    look as expected."""
    blk = nc.main_func.blocks[0]
    dma = [i for i in blk.instructions if isinstance(i, mybir.InstDMACopy)]
    wait = [i for i in blk.instructions
            if isinstance(i, mybir.InstEventSemaphore) and "dma_sem" in str(i)]
    memsets = [i for i in blk.instructions if isinstance(i, mybir.InstMemset)]
    if len(dma) != 1 or len(wait) != 1 or not memsets:
        return
    drop = {id(i) for i in memsets} | {id(dma[0]), id(wait[0])}
    rest = [i for i in blk.instructions if id(i) not in drop]
    blk.instructions[:] = rest[:1] + dma + wait + rest[1:] + memsets[:1]


def _build():
    nc = bass.Bass(enable_partition_id=False, monotonic_sem_count=0)
    fgcn_b = nc.declare_dram_parameter("fgcn_b", [S], mybir.dt.float32, isOutput=False)
    out = nc.declare_dram_parameter("out", [B_PER, S], mybir.dt.float32, isOutput=True)
    with nc.semaphore("dma_sem") as dma_sem:
        nc.sync.dma_start(
            out=out[:, :], in_=fgcn_b[None, :].broadcast_to((B_PER, S))
        ).then_inc(dma_sem, 16)
        nc.gpsimd.wait_ge(dma_sem, 16)
    try:
        _reschedule(nc)
    except Exception:
        pass
    nc.finalize()
    return nc


def _subprocess_retry(fgcn_b: np.ndarray) -> np.ndarray:
    with tempfile.TemporaryDirectory() as td:
        inp = os.path.join(td, "in.npy")
        outp = os.path.join(td, "out.npy")
        np.save(inp, fgcn_b)
        code = (
            "import sys, numpy as np\n"
            f"sys.path.insert(0, {os.path.dirname(os.path.abspath(__file__))!r})\n"
            "import kernel\n"
            f"out = kernel.kernel(fgcn_b=np.load({inp!r}))\n"
            f"np.save({outp!r}, out)\n"
        )
        env = dict(os.environ)
        env["KERNEL_NO_SUBPROCESS"] = "1"
        env.pop("KERNEL_TRACE", None)
        subprocess.run([sys.executable, "-c", code], check=True, env=env, timeout=900)
        return np.load(outp)


def kernel(**inputs) -> np.ndarray:
    global LAST_RESULT
    nc = _CACHE.get("nc")
    if nc is None:
        nc = _CACHE["nc"] = _build()
    fgcn_b = np.ascontiguousarray(np.asarray(inputs["fgcn_b"], dtype=np.float32))
    assert fgcn_b.shape == (S,), fgcn_b.shape
    in_maps = [{"fgcn_b": fgcn_b} for _ in range(N_CORES)]
    trace = os.environ.get("KERNEL_TRACE", "") == "1"
    res = None
    last_err = None
    for _attempt in range(3):
        try:
            res = run_bass_kernel_spmd(nc, in_maps, list(range(N_CORES)), trace=trace)
            break
        except ModuleNotFoundError:
            # Tracing was requested (possibly via BASS_TRACE in the
            # environment) but the axon NTFF profile hook module is
            # unavailable here — rerun with tracing forced off.
            os.environ["BASS_NEVER_TRACE"] = "1"
            trace = False
        except Exception as e:  # transient device wedge (e.g. NRT_EXEC_UNIT_UNRECOVERABLE)
            last_err = e
    if res is None:
        # A wedged device poisons the whole PJRT session; a fresh process
        # (fresh axon session + device open) typically succeeds. Retry there
        # once unless we already are such a retry.
        if os.environ.get("KERNEL_NO_SUBPROCESS") == "1":
            raise last_err
        return _subprocess_retry(fgcn_b)
    LAST_RESULT = res
    shards = [res.results[i]["out"] for i in range(N_CORES)]
    return np.concatenate(shards, axis=0).reshape(B, S, 1)
